# revision 24
# baseline (speedup 1.0000x reference)
"""Trainium2 Bass kernel for nn_ASG_seq2seq_quantile_onehot.

Data-parallel over the 8 graphs (batch is sorted -> graphs are contiguous row
ranges).  Each of the 8 NeuronCores processes one graph (padded to NP=1152
nodes):
  - KNN (K=8) on-device in fp32 via Max8 + match_replace (matches jax top_k
    tie-breaking: value-descending, lowest index first).
  - Softmax edge weights -> dense [NP,NP] weight matrix W, transposed on the
    PE array; message passing agg = W @ m implemented as dense matmuls.
    (mask input is all-ones per the spec, so the edge set is horizon-invariant)
  - Encoder LSTM (24 steps, hidden 768) / decoder LSTM (12 steps) with the
    hidden state kept transposed [768, NP]; weights in bf16, gate accumulation
    in fp32 PSUM, cell state fp32.
  - The hidden dimension is host-permuted (r' = t*64 + j  <->  old j*12 + t) so
    the per-step zcat h-slice is a contiguous 64-partition block.
All heavy math runs on the NeuronCores; the host only gathers the attribute
embeddings (s_l, also an output), repacks/transposes weights, shards, pads and
unshards.
"""

import numpy as np

import concourse.bass as bass
import concourse.bacc as bacc
import concourse.mybir as mybir
from concourse.tile import TileContext
from concourse.bass_utils import run_bass_kernel_spmd

F32 = mybir.dt.float32
BF16 = mybir.dt.bfloat16
AF = mybir.ActivationFunctionType
OP = mybir.AluOpType

N, F, T, H = 8192, 32, 24, 12
HSS = 768
G = 16
K = 8
NQ = 9
ZF = 95
NP = 1152          # per-core node capacity (9 x 128)
NCHUNK = NP // 128  # 9
NCORES = 8
NT = 3             # free-dim tiles of 384
FT = NP // NT      # 384
QC = 24            # 3072/128 gate chunks
CC = 6             # 768/128 hidden chunks

_PROG_CACHE = {}
ENC_STEPS = T


def _perm():
    return np.array([(r % 64) * 12 + r // 64 for r in range(HSS)], dtype=np.int64)


def build_program():
    nc = bacc.Bacc("TRN2", target_bir_lowering=False, debug=False)

    def inp(name, shape, dt=BF16):
        return nc.declare_dram_parameter(name, list(shape), dt, isOutput=False)

    slT_d = inp("slT", [4, NP], F32)
    xT_d = inp("xT", [T * F, NP])          # [t*32+f, n] bf16
    zT_d = inp("zT", [31, H * NP])         # [c, t*NP+n] bf16
    whhE_d = inp("whhE", [HSS, 4 * HSS])
    wihE_d = inp("wihE", [F, 4 * HSS])
    ebias_d = inp("ebias", [128, QC], F32)
    whhD_d = inp("whhD", [HSS, 4 * HSS])
    wihDxz_d = inp("wihDxz", [ZF, 4 * HSS])
    wihDg_d = inp("wihDg", [G, 4 * HSS])
    dbias_d = inp("dbias", [128, QC], F32)
    lin0T_d = inp("lin0T", [HSS, HSS])
    lin0b_d = inp("lin0b", [128, CC], F32)
    WzTh_d = inp("WzTh", [64, ZF])
    WzTz_d = inp("WzTz", [31, ZF])
    bz_d = inp("bz", [ZF, 1], F32)
    WxT_d = inp("WxT", [F, F])
    bx_d = inp("bx", [F, 1], F32)
    w1T_d = inp("w1T", [HSS, 384])
    b1_d = inp("b1", [128, 3], F32)
    w2T_d = inp("w2T", [3 * 128, NQ])
    b2_d = inp("b2", [NQ, 1], F32)
    gwihT_d = inp("gwihT", [G, 3 * G])
    gwhhT_d = inp("gwhhT", [G, 3 * G])
    gbi_d = inp("gbi", [3 * G, 1], F32)
    gbiz_d = inp("gbiz", [G, 1], F32)
    gbh_d = inp("gbh", [3 * G, 1], F32)
    ggc_d = inp("ggc", [3 * G, G])

    pred_d = nc.declare_dram_parameter("pred", [H * NQ, NP], F32, isOutput=True)
    wdump_d = nc.declare_dram_parameter("wdump", [NCHUNK * 128, NP], F32, isOutput=True)
    hdump_d = nc.declare_dram_parameter("hdump", [HSS, NP], F32, isOutput=True)
    h2dump_d = nc.declare_dram_parameter("h2dump", [HSS, NP], F32, isOutput=True)

    with TileContext(nc) as tc:
        # ---------------- persistent pools ----------------
        _drcm = tc.tile_pool(name="drsc", bufs=1, space="DRAM")
        drp = _drcm.__enter__()
        xe_s = drp.tile([T * F, NP], BF16, name="xe_s")
        xz_s = drp.tile([H * ZF, NP], BF16, name="xz_s")
        g_s = drp.tile([H * G, NP], BF16, name="g_s")
        _wcm = tc.tile_pool(name="w", bufs=1)
        wpool = _wcm.__enter__()

        # ---------------- KNN phase (fp32) ----------------
        WT_sb = [wpool.tile([128, NP], BF16, name=f"WT{j}", tag=f"WT{j}")
                 for j in range(NCHUNK)]
        with tc.tile_pool(name="knn", bufs=1) as kp, \
             tc.tile_pool(name="knn2", bufs=2) as kp2, \
             tc.tile_pool(name="knnps", bufs=2, space="PSUM") as pp:
            slT = kp.tile([4, NP], F32)
            nc.sync.dma_start(out=slT[:], in_=slT_d.ap())
            sl2T = kp.tile([4, NP], F32)
            nc.vector.tensor_scalar_mul(sl2T[:], slT[:], 2.0)
            sq4 = kp.tile([4, NP], F32)
            nc.vector.tensor_mul(sq4[:], slT[:], slT[:])
            ones4 = kp.tile([4, 1], F32)
            nc.vector.memset(ones4[:], 1.0)
            neg1 = kp.tile([1, 128], F32)
            nc.vector.memset(neg1[:], -1.0)
            sq_row = kp.tile([1, NP], F32)
            for nt in range(NT):
                ps = pp.tile([1, FT], F32, tag="sqr")
                nc.tensor.matmul(ps[:], ones4[:], sq4[:, nt * FT:(nt + 1) * FT],
                                 start=True, stop=True)
                nc.vector.tensor_copy(sq_row[:, nt * FT:(nt + 1) * FT], ps[:])
            sqc = kp.tile([128, NCHUNK], F32)
            for ic in range(NCHUNK):
                ps = pp.tile([128, 1], F32, tag="sqc")
                nc.tensor.matmul(ps[:], sq4[:, ic * 128:(ic + 1) * 128], ones4[:],
                                 start=True, stop=True)
                nc.vector.tensor_copy(sqc[:, ic:ic + 1], ps[:])

            ident = kp.tile([128, 128], F32)
            iot_r = kp.tile([128, 128], F32)
            nc.gpsimd.iota(iot_r[:], pattern=[[1, 128]], base=0, channel_multiplier=0,
                           allow_small_or_imprecise_dtypes=True)
            iot_c = kp.tile([128, 1], F32)
            nc.gpsimd.iota(iot_c[:], pattern=[[0, 1]], base=0, channel_multiplier=1,
                           allow_small_or_imprecise_dtypes=True)
            nc.vector.tensor_scalar(ident[:], iot_r[:], iot_c[:], None, op0=OP.is_equal)

            for ic in range(NCHUNK):
                Srow = kp2.tile([128, NP], F32, tag="Srow")
                for nt in range(NT):
                    ps = pp.tile([128, FT], F32, tag="Sps")
                    nc.tensor.matmul(ps[:], sl2T[:, ic * 128:(ic + 1) * 128],
                                     slT[:, nt * FT:(nt + 1) * FT],
                                     start=True, stop=False)
                    nc.tensor.matmul(ps[:], neg1[:],
                                     sq_row[:, nt * FT:(nt + 1) * FT],
                                     start=False, stop=True)
                    nc.vector.tensor_copy(Srow[:, nt * FT:(nt + 1) * FT], ps[:])
                s8 = kp2.tile([128, 8], F32, tag="s8")
                nc.vector.max(out=s8[:], in_=Srow[:])
                smk = kp2.tile([128, NP], F32, tag="smk")
                nc.vector.match_replace(out=smk[:], in_to_replace=s8[:],
                                        in_values=Srow[:], imm_value=-1e30)
                msk = kp2.tile([128, NP], F32, tag="msk")
                nc.vector.tensor_scalar(msk[:], smk[:], -1e29, None, op0=OP.is_le)
                bcol = kp2.tile([128, 1], F32, tag="bcol")
                nc.vector.tensor_scalar_mul(bcol[:], sqc[:, ic:ic + 1], -10.0)
                ew8 = kp2.tile([128, 8], F32, tag="ew8")
                nc.scalar.activation(ew8[:], s8[:], AF.Exp, bias=bcol[:], scale=10.0)
                e2 = kp2.tile([128, 8], F32, tag="e2")
                nc.scalar.activation(e2[:], ew8[:], AF.Exp)
                zsum = kp2.tile([128, 1], F32, tag="zs")
                nc.vector.reduce_sum(zsum[:], e2[:], axis=mybir.AxisListType.X)
                rz = kp2.tile([128, 1], F32, tag="rz")
                nc.vector.reciprocal(rz[:], zsum[:])
                # reuse smk as scratch for clamped S -> E -> E2
                nc.vector.tensor_scalar_min(smk[:], Srow[:], 2.0)
                nc.scalar.activation(smk[:], smk[:], AF.Exp, bias=bcol[:], scale=10.0)
                nc.scalar.activation(smk[:], smk[:], AF.Exp)
                nc.vector.tensor_scalar(smk[:], smk[:], rz[:], None, op0=OP.mult)
                Wrow = kp2.tile([128, NP], F32, tag="Wrow")
                nc.vector.tensor_mul(Wrow[:], smk[:], msk[:])
                nc.sync.dma_start(out=wdump_d.ap()[ic * 128:(ic + 1) * 128, :],
                                  in_=Wrow[:])
                for jc in range(NCHUNK):
                    ps = pp.tile([128, 128], F32, tag="Tps")
                    nc.tensor.transpose(ps[:], Wrow[:, jc * 128:(jc + 1) * 128],
                                        ident[:])
                    nc.vector.tensor_copy(WT_sb[jc][:, ic * 128:(ic + 1) * 128], ps[:])

        # ---------------- load weights ----------------
        def wload(dram, shape, dt=BF16, tag=None):
            t_ = wpool.tile(list(shape), dt, tag=tag or dram.name)
            nc.sync.dma_start(out=t_[:], in_=dram.ap())
            return t_

        whhE = [wpool.tile([128, 4 * HSS], BF16, name=f"whhE{c}", tag=f"whhE{c}") for c in range(CC)]
        for c in range(CC):
            nc.sync.dma_start(out=whhE[c][:], in_=whhE_d.ap()[c * 128:(c + 1) * 128, :])
        wihE = wload(wihE_d, [F, 4 * HSS])
        eb = wload(ebias_d, [128, QC], F32)
        whhD = [wpool.tile([128, 4 * HSS], BF16, name=f"whhD{c}", tag=f"whhE{c}") for c in range(CC)]
        for c in range(CC):
            nc.sync.dma_start(out=whhD[c][:], in_=whhD_d.ap()[c * 128:(c + 1) * 128, :])
        wihDxz = wload(wihDxz_d, [ZF, 4 * HSS])
        wihDg = wload(wihDg_d, [G, 4 * HSS])
        db = wload(dbias_d, [128, QC], F32)
        lin0T = [wpool.tile([128, HSS], BF16, name=f"l0{c}", tag=f"l0{c}") for c in range(CC)]
        for c in range(CC):
            nc.sync.dma_start(out=lin0T[c][:], in_=lin0T_d.ap()[c * 128:(c + 1) * 128, :])
        lin0b = wload(lin0b_d, [128, CC], F32)
        WzTh = wload(WzTh_d, [64, ZF])
        WzTz = wload(WzTz_d, [31, ZF])
        bzc = wload(bz_d, [ZF, 1], F32)
        WxT = wload(WxT_d, [F, F])
        bxc = wload(bx_d, [F, 1], F32)
        w1T = [wpool.tile([128, 384], BF16, name=f"w1T{c}", tag=f"w1T{c}") for c in range(CC)]
        for c in range(CC):
            nc.sync.dma_start(out=w1T[c][:], in_=w1T_d.ap()[c * 128:(c + 1) * 128, :])
        b1c = wload(b1_d, [128, 3], F32)
        w2T = [wpool.tile([128, NQ], BF16, name=f"w2T{c}", tag=f"w2T{c}") for c in range(3)]
        for c in range(3):
            nc.sync.dma_start(out=w2T[c][:], in_=w2T_d.ap()[c * 128:(c + 1) * 128, :])
        b2c = wload(b2_d, [NQ, 1], F32)
        gwihT = wload(gwihT_d, [G, 3 * G])
        gwhhT = wload(gwhhT_d, [G, 3 * G])
        gbi = wload(gbi_d, [3 * G, 1], F32)
        gbiz = wload(gbiz_d, [G, 1], F32)
        gbh = wload(gbh_d, [3 * G, 1], F32)
        ggc = [wpool.tile([G, G], BF16, name=f"ggc{l}", tag=f"ggc{l}") for l in range(3)]
        for l in range(3):
            nc.sync.dma_start(out=ggc[l][:], in_=ggc_d.ap()[l * G:(l + 1) * G, :])

        # state tiles
        _scm = tc.tile_pool(name="state", bufs=1)
        spool = _scm.__enter__()
        hTb = [spool.tile([128, NP], BF16, name=f"h{c}", tag=f"h{c}") for c in range(CC)]
        cT = [spool.tile([128, NP], F32, name=f"c{c}", tag=f"c{c}") for c in range(CC)]
        for c in range(CC):
            nc.vector.memset(hTb[c][:], 0.0)
            nc.vector.memset(cT[c][:], 0.0)
        sg = [spool.tile([128, NP], BF16, name=f"sg{g_}", tag=f"sg{g_}") for g_ in range(4)]
        tmp1 = spool.tile([128, NP], F32, tag="tmp1")
        hOld = [spool.tile([128, NP], BF16, name=f"hO{c}", tag=f"hO{c}")
                for c in range(CC)]

        # ---------------- xe precompute ----------------
        _xecm = tc.tile_pool(name="xeps", bufs=2, space="PSUM")
        xe_pp = _xecm.__enter__()
        for t in range(T):
            xstg = spool.tile([F, NP], BF16, tag="xstg", bufs=2)
            nc.sync.dma_start(out=xstg[:], in_=xT_d.ap()[t * F:(t + 1) * F, :])
            xet = spool.tile([F, NP], BF16, tag="xet", bufs=2)
            for nt in range(NT):
                ps = xe_pp.tile([F, FT], F32, tag="xeps")
                nc.tensor.matmul(ps[:], WxT[:],
                                 xstg[:, nt * FT:(nt + 1) * FT],
                                 start=True, stop=True)
                nc.scalar.activation(xet[:, nt * FT:(nt + 1) * FT], ps[:],
                                     AF.Tanh, bias=bxc[:], scale=1.0)
            nc.sync.dma_start(out=xe_s[t * F:(t + 1) * F, :], in_=xet[:])
        _xecm.__exit__(None, None, None)

        # ---------------- LSTM step helper ----------------
        def lstm_step(whh, wih_list, rhs_list, bias, pool):
            """wih_list: [(lhsT_tile, rhs_tile, rhs_off_expr)]; gates into sg, update cT/hTb."""
            for c in range(CC):
                nc.vector.tensor_copy(hOld[c][:], hTb[c][:])
            for hc in range(CC):
                for gate in range(4):
                    q = gate * CC + hc
                    for nt in range(NT):
                        ps = pool.tile([128, FT], F32, tag="gps")
                        for c in range(CC):
                            nc.tensor.matmul(
                                ps[:], whh[c][:, q * 128:(q + 1) * 128],
                                hOld[c][:, nt * FT:(nt + 1) * FT],
                                start=(c == 0), stop=False)
                        nmm = len(wih_list)
                        for wi, (lt, rt, roff) in enumerate(wih_list):
                            nc.tensor.matmul(
                                ps[:], lt[:, q * 128:(q + 1) * 128],
                                rt[:, bass.ds(roff + nt * FT, FT)] if roff is not None
                                else rt[:, nt * FT:(nt + 1) * FT],
                                start=False, stop=(wi == nmm - 1))
                        fn = AF.Tanh if gate == 2 else AF.Sigmoid
                        nc.scalar.activation(sg[gate][:, nt * FT:(nt + 1) * FT], ps[:],
                                             fn, bias=bias[:, q:q + 1], scale=1.0)
                nc.vector.tensor_mul(cT[hc][:], sg[1][:], cT[hc][:])
                nc.vector.tensor_mul(tmp1[:], sg[0][:], sg[2][:])
                nc.vector.tensor_add(cT[hc][:], cT[hc][:], tmp1[:])
                nc.scalar.activation(tmp1[:], cT[hc][:], AF.Tanh)
                nc.vector.tensor_mul(hTb[hc][:], sg[3][:], tmp1[:])

        # ---------------- encoder loop ----------------
        _ecm = tc.tile_pool(name="encps", bufs=4, space="PSUM")
        enc_pp = _ecm.__enter__()
        with tc.For_i(0, ENC_STEPS, 1, hint_engines=(mybir.EngineType.PE,)) as it:
            xet = spool.tile([F, NP], BF16, tag="xet", bufs=2)
            nc.sync.dma_start(out=xet[:], in_=xe_s[bass.ds(it * F, F), :])
            lstm_step(whhE, [(wihE, xet, None)], None, eb, enc_pp)

        for c in range(CC):
            nc.gpsimd.dma_start(out=hdump_d.ap()[c * 128:(c + 1) * 128, :],
                                in_=hTb[c][:])
        # ---------------- lin0 (into hOld, reused as h2) ----------------
        h2Tb = hOld
        for oc in range(CC):
            for nt in range(NT):
                ps = enc_pp.tile([128, FT], F32, tag="gps")
                for c in range(CC):
                    nc.tensor.matmul(ps[:], lin0T[c][:, oc * 128:(oc + 1) * 128],
                                     hTb[c][:, nt * FT:(nt + 1) * FT],
                                     start=(c == 0), stop=(c == CC - 1))
                nc.scalar.add(tmp1[:, nt * FT:(nt + 1) * FT], ps[:],
                              lin0b[:, oc:oc + 1])
            nc.vector.scalar_tensor_tensor(h2Tb[oc][:], tmp1[:], 0.01, tmp1[:],
                                           op0=OP.mult, op1=OP.max)

        _ecm.__exit__(None, None, None)

        for c in range(CC):
            nc.gpsimd.dma_start(out=h2dump_d.ap()[c * 128:(c + 1) * 128, :],
                                in_=h2Tb[c][:])
        # ---------------- xz precompute (12 t) ----------------
        _xzcm = tc.tile_pool(name="xzps", bufs=2, space="PSUM")
        xz_pp = _xzcm.__enter__()
        for t in range(H):
            xzt = spool.tile([ZF, NP], BF16, tag="xzt")
            hstg = spool.tile([64, NP], BF16, tag="hstg")
            po = (t % 2) * 64
            nc.sync.dma_start(out=hstg[:], in_=h2Tb[t // 2][po:po + 64, :])
            zstg = spool.tile([31, NP], BF16, tag="zstg", bufs=2)
            nc.sync.dma_start(out=zstg[:], in_=zT_d.ap()[:, t * NP:(t + 1) * NP])
            for nt in range(NT):
                ps = xz_pp.tile([ZF, FT], F32, tag="xzps")
                nc.tensor.matmul(ps[:], WzTh[:],
                                 hstg[:, nt * FT:(nt + 1) * FT],
                                 start=True, stop=False)
                nc.tensor.matmul(ps[:], WzTz[:],
                                 zstg[:, nt * FT:(nt + 1) * FT],
                                 start=False, stop=True)
                nc.scalar.activation(xzt[:, nt * FT:(nt + 1) * FT], ps[:],
                                     AF.Tanh, bias=bzc[:], scale=1.0)
            nc.sync.dma_start(out=xz_s[t * ZF:(t + 1) * ZF, :], in_=xzt[:])

        _xzcm.__exit__(None, None, None)
        # decoder state: h <- lin0 output (currently in hOld/h2Tb)
        for c in range(CC):
            nc.vector.tensor_copy(hTb[c][:], h2Tb[c][:])

        # ---------------- GNN (12 t) ----------------
        _gpscm = tc.tile_pool(name="gnnps", bufs=1, space="PSUM")
        gnn_pp = _gpscm.__enter__()
        _gnncm = tc.tile_pool(name="gnn", bufs=1)
        gnp = _gnncm.__enter__()
        hgT = gnp.tile([G, NP], BF16, tag="hgT")
        m_sb = [gnp.tile([128, G], BF16, name=f"m{j}", tag=f"m{j}") for j in range(NCHUNK)]
        aggT = gnp.tile([G, NP], BF16, tag="aggT")
        rT = gnp.tile([G, NP], BF16, tag="rT")
        zTt = gnp.tile([G, NP], BF16, tag="zTt")
        ginT = gnp.tile([G, NP], BF16, tag="ginT")
        ghnT = gnp.tile([G, NP], BF16, tag="ghnT")
        nnT = gnp.tile([G, NP], BF16, tag="nnT")
        t5 = gnp.tile([G, NP], BF16, tag="t5")
        with tc.For_i(0, H, 1, hint_engines=(mybir.EngineType.PE,)) as it:
            nc.vector.memset(hgT[:], 0.0)
            nc.sync.dma_start(out=hgT[0:4, :], in_=zT_d.ap()[0:4, bass.ds(it * NP, NP)])
            for l in range(3):
                for jc in range(NCHUNK):
                    ps = gnn_pp.tile([128, G], F32, tag="mps")
                    nc.tensor.matmul(ps[:], hgT[:, jc * 128:(jc + 1) * 128],
                                     ggc[l][:], start=True, stop=True)
                    nc.vector.tensor_copy(m_sb[jc][:], ps[:])
                for nt in range(NT):
                    ps = gnn_pp.tile([G, FT], F32, tag="aps")
                    for jc in range(NCHUNK):
                        nc.tensor.matmul(ps[:], m_sb[jc][:],
                                         WT_sb[jc][:, nt * FT:(nt + 1) * FT],
                                         start=(jc == 0), stop=(jc == NCHUNK - 1))
                    nc.vector.tensor_copy(aggT[:, nt * FT:(nt + 1) * FT], ps[:])
                for nt in range(NT):
                    sl_ = slice(nt * FT, (nt + 1) * FT)
                    psr = gnn_pp.tile([G, FT], F32, tag="rps")
                    nc.tensor.matmul(psr[:], gwihT[:, 0:G], aggT[:, sl_],
                                     start=True, stop=False)
                    nc.tensor.matmul(psr[:], gwhhT[:, 0:G], hgT[:, sl_],
                                     start=False, stop=True)
                    nc.scalar.activation(rT[:, sl_], psr[:], AF.Sigmoid,
                                         bias=gbi[0:G, :], scale=1.0)
                    psz = gnn_pp.tile([G, FT], F32, tag="zps")
                    nc.tensor.matmul(psz[:], gwihT[:, G:2 * G], aggT[:, sl_],
                                     start=True, stop=False)
                    nc.tensor.matmul(psz[:], gwhhT[:, G:2 * G], hgT[:, sl_],
                                     start=False, stop=True)
                    nc.scalar.activation(zTt[:, sl_], psz[:], AF.Sigmoid,
                                         bias=gbiz[:], scale=1.0)
                    ps2 = gnn_pp.tile([G, FT], F32, tag="inps")
                    nc.tensor.matmul(ps2[:], gwihT[:, 2 * G:3 * G], aggT[:, sl_],
                                     start=True, stop=True)
                    nc.scalar.add(ginT[:, sl_], ps2[:], gbi[2 * G:3 * G, :])
                    ps3 = gnn_pp.tile([G, FT], F32, tag="hnps")
                    nc.tensor.matmul(ps3[:], gwhhT[:, 2 * G:3 * G], hgT[:, sl_],
                                     start=True, stop=True)
                    nc.scalar.add(ghnT[:, sl_], ps3[:], gbh[2 * G:3 * G, :])
                # note: r/z bias uses gbi only; add gbh contribution via matmul?  The
                # reference computes sigmoid(ir+hr+bih_r+bhh_r).  gbh_r is folded on
                # the host into gbi rows 0:2G.
                nc.vector.tensor_mul(t5[:], rT[:], ghnT[:])
                nc.vector.tensor_add(ginT[:], ginT[:], t5[:])
                nc.scalar.activation(nnT[:], ginT[:], AF.Tanh)
                nc.vector.tensor_sub(t5[:], hgT[:], nnT[:])
                nc.vector.tensor_mul(t5[:], zTt[:], t5[:])
                nc.vector.tensor_add(hgT[:], nnT[:], t5[:])
            gt = gnp.tile([G, NP], BF16, tag="nnT")
            nc.vector.scalar_tensor_tensor(gt[:], hgT[:], 0.01, hgT[:],
                                           op0=OP.mult, op1=OP.max)
            nc.sync.dma_start(out=g_s[bass.ds(it * G, G), :], in_=gt[:])
        _gnncm.__exit__(None, None, None)
        _gpscm.__exit__(None, None, None)

        # ---------------- decoder loop ----------------
        _dcm = tc.tile_pool(name="decps", bufs=4, space="PSUM")
        dec_pp = _dcm.__enter__()
        y1b = [spool.tile([128, NP], BF16, name=f"y1b{c}", tag=f"y1b{c}") for c in range(3)]
        predt = spool.tile([NQ, NP], F32, tag="predt")  # small
        with tc.For_i(0, H, 1, hint_engines=(mybir.EngineType.PE,)) as it:
            xzt = spool.tile([ZF, NP], BF16, tag="xzt")
            nc.sync.dma_start(out=xzt[:], in_=xz_s[bass.ds(it * ZF, ZF), :])
            gt = spool.tile([G, NP], BF16, tag="gt")
            nc.sync.dma_start(out=gt[:], in_=g_s[bass.ds(it * G, G), :])
            lstm_step(whhD, [(wihDxz, xzt, None), (wihDg, gt, None)], None, db, dec_pp)
            # head
            for oc in range(3):
                for nt in range(NT):
                    ps = dec_pp.tile([128, FT], F32, tag="gps")
                    for c in range(CC):
                        nc.tensor.matmul(ps[:], w1T[c][:, oc * 128:(oc + 1) * 128],
                                         hTb[c][:, nt * FT:(nt + 1) * FT],
                                         start=(c == 0), stop=(c == CC - 1))
                    nc.scalar.add(tmp1[:, nt * FT:(nt + 1) * FT], ps[:],
                                  b1c[:, oc:oc + 1])
                nc.vector.scalar_tensor_tensor(y1b[oc][:], tmp1[:], 0.01, tmp1[:],
                                               op0=OP.mult, op1=OP.max)
            for nt in range(NT):
                ps = dec_pp.tile([NQ, FT], F32, tag="y2ps", bufs=2)
                for oc in range(3):
                    nc.tensor.matmul(ps[:], w2T[oc][:],
                                     y1b[oc][:, nt * FT:(nt + 1) * FT],
                                     start=(oc == 0), stop=(oc == 2))
                nc.scalar.add(predt[:, nt * FT:(nt + 1) * FT], ps[:], b2c[:])
            nc.vector.scalar_tensor_tensor(predt[:], predt[:], 0.1, predt[:],
                                           op0=OP.mult, op1=OP.max)
            nc.sync.dma_start(out=pred_d.ap()[bass.ds(it * NQ, NQ), :], in_=predt[:])

        _dcm.__exit__(None, None, None)
        _scm.__exit__(None, None, None)
        _wcm.__exit__(None, None, None)
        _drcm.__exit__(None, None, None)

    nc.compile()
    return nc


def _prep_host(inputs):
    f32 = np.float32
    x = np.asarray(inputs["x"], f32)
    f = np.asarray(inputs["f"])
    batch = np.asarray(inputs["batch"])
    z = np.asarray(inputs["z"], f32)
    wa = [np.asarray(inputs[f"wa{i}"], f32) for i in range(4)]

    s_l = np.tanh(np.stack([wa[0][f[:, 0]], wa[1][f[:, 1]],
                            wa[2][f[:, 2]], wa[3][f[:, 3]]], axis=-1)).astype(f32)

    starts = np.searchsorted(batch, np.arange(NCORES))
    ends = np.searchsorted(batch, np.arange(NCORES), side="right")
    sizes = ends - starts
    assert sizes.max() <= NP, f"graph too large: {sizes.max()}"

    perm = _perm()
    gperm = np.concatenate([gi * HSS + perm for gi in range(4)])

    import ml_dtypes
    def tobf(a):
        return np.asarray(a, dtype=ml_dtypes.bfloat16)

    w = {k: np.asarray(inputs[k], f32) for k in (
        "Wx", "bx", "Wz", "bz", "ggc_w", "gru_wih", "gru_whh", "gru_bih", "gru_bhh",
        "enc_wih", "enc_whh", "enc_bih", "enc_bhh",
        "dec_wih", "dec_whh", "dec_bih", "dec_bhh",
        "lin0_w", "lin0_b", "out_w1", "out_b1", "out_w2", "out_b2")}

    shared = {}
    shared["WxT"] = tobf(np.ascontiguousarray(w["Wx"].T))
    shared["bx"] = w["bx"][:, None]
    ewih = w["enc_wih"][gperm]
    shared["wihE"] = tobf(np.ascontiguousarray(ewih.T))
    ewhh = w["enc_whh"][gperm][:, perm]
    shared["whhE"] = tobf(np.ascontiguousarray(ewhh.T))
    shared["ebias"] = np.ascontiguousarray(
        (w["enc_bih"] + w["enc_bhh"])[gperm].reshape(QC, 128).T)
    dwih = w["dec_wih"][gperm]
    shared["wihDxz"] = tobf(np.ascontiguousarray(dwih[:, 0:ZF].T))
    shared["wihDg"] = tobf(np.ascontiguousarray(dwih[:, ZF:ZF + G].T))
    dwhh = w["dec_whh"][gperm][:, perm]
    shared["whhD"] = tobf(np.ascontiguousarray(dwhh.T))
    shared["dbias"] = np.ascontiguousarray(
        (w["dec_bih"] + w["dec_bhh"])[gperm].reshape(QC, 128).T)
    l0 = w["lin0_w"][perm][:, perm]
    shared["lin0T"] = tobf(np.ascontiguousarray(l0.T))
    shared["lin0b"] = np.ascontiguousarray(w["lin0_b"][perm].reshape(CC, 128).T)
    WzT = np.ascontiguousarray(w["Wz"].T)  # [c, q]
    shared["WzTh"] = tobf(WzT[0:64])
    shared["WzTz"] = tobf(WzT[64:ZF])
    shared["bz"] = w["bz"][:, None]
    w1p = w["out_w1"][:, perm]
    shared["w1T"] = tobf(np.ascontiguousarray(w1p.T))
    shared["b1"] = np.ascontiguousarray(w["out_b1"].reshape(3, 128).T)
    shared["w2T"] = tobf(np.ascontiguousarray(w["out_w2"].T))
    shared["b2"] = w["out_b2"][:, None]
    shared["gwihT"] = tobf(np.ascontiguousarray(w["gru_wih"].T))
    shared["gwhhT"] = tobf(np.ascontiguousarray(w["gru_whh"].T))
    gbi = (w["gru_bih"]).copy()
    gbi[0:2 * G] += w["gru_bhh"][0:2 * G]   # fold bhh_{r,z} into gbi
    shared["gbi"] = gbi[:, None]
    shared["gbiz"] = gbi[G:2 * G, None]
    shared["gbh"] = w["gru_bhh"][:, None]
    shared["ggc"] = tobf(np.ascontiguousarray(w["ggc_w"].reshape(3 * G, G)))

    in_maps = []
    for g_ in range(NCORES):
        s, e = starts[g_], ends[g_]
        ng = e - s
        m = dict(shared)
        slT = np.full((4, NP), 1e3, f32)
        slT[:, :ng] = s_l[s:e].T
        m["slT"] = slT
        xT = np.zeros((T * F, NP), f32)
        xT.reshape(T, F, NP)[:, :, :ng] = x[s:e].transpose(2, 1, 0)
        m["xT"] = tobf(xT)
        zTl = np.zeros((31, H * NP), f32)
        zTl.reshape(31, H, NP)[:, :, :ng] = z[s:e].transpose(1, 2, 0)
        m["zT"] = tobf(zTl)
        in_maps.append(m)
    return in_maps, starts, ends, s_l


def _get_runner():
    """Build the Bass program once and wrap it in a cached sharded jit so
    repeat kernel() calls skip jax re-tracing."""
    if "runner" in _PROG_CACHE:
        return _PROG_CACHE["runner"]
    import jax
    from jax.sharding import Mesh, PartitionSpec
    from jax.experimental.shard_map import shard_map
    from concourse import bass2jax
    import concourse.mybir as mb

    nc = build_program()
    bass2jax.install_neuronx_cc_hook()
    partition_name = nc.partition_id_tensor.name if nc.partition_id_tensor else None
    in_names, out_names, out_avals, zero_shapes = [], [], [], []
    for alloc in nc.m.functions[0].allocations:
        if not isinstance(alloc, mb.MemoryLocationSet):
            continue
        name = alloc.memorylocations[0].name
        if alloc.kind == "ExternalInput":
            if name != partition_name:
                in_names.append(name)
        elif alloc.kind == "ExternalOutput":
            out_names.append(name)
            shape = tuple(alloc.tensor_shape)
            dtype = mb.dt.np(alloc.dtype)
            out_avals.append(jax.core.ShapedArray(shape, dtype))
            zero_shapes.append((shape, dtype))
    n_params = len(in_names)
    full_in_names = list(in_names) + list(out_names)
    if partition_name is not None:
        full_in_names.append(partition_name)
    donate = tuple(range(n_params, n_params + len(out_names)))

    def _body(*args):
        operands = list(args)
        if partition_name is not None:
            operands.append(bass2jax.partition_id_tensor())
        outs = bass2jax._bass_exec_p.bind(
            *operands,
            out_avals=tuple(out_avals),
            in_names=tuple(full_in_names),
            out_names=tuple(out_names),
            lowering_input_output_aliases=(),
            sim_require_finite=True,
            sim_require_nnan=True,
            nc=nc,
        )
        return tuple(outs)

    devices = jax.devices()[:NCORES]
    mesh = Mesh(np.asarray(devices), ("core",))
    in_specs = (PartitionSpec("core"),) * (n_params + len(out_names))
    out_specs = (PartitionSpec("core"),) * len(out_names)
    sharded = jax.jit(
        shard_map(_body, mesh=mesh, in_specs=in_specs, out_specs=out_specs,
                  check_rep=False),
        donate_argnums=donate, keep_unused=True)

    def run(in_maps):
        concat_in = [np.concatenate([in_maps[c][nm] for c in range(NCORES)], axis=0)
                     for nm in in_names]
        concat_zeros = [np.zeros((NCORES * sh[0], *sh[1:]), dt)
                        for sh, dt in zero_shapes]
        out_arrs = sharded(*concat_in, *concat_zeros)
        return {nm: np.asarray(out_arrs[i]).reshape(NCORES, *out_avals[i].shape)
                for i, nm in enumerate(out_names)}

    _PROG_CACHE["runner"] = run
    return run


def kernel(**inputs):
    run = _get_runner()
    in_maps, starts, ends, s_l = _prep_host(inputs)
    outs = run(in_maps)
    pred = np.zeros((N, NQ, H), np.float32)
    for g_ in range(NCORES):
        s, e = starts[g_], ends[g_]
        pt = outs["pred"][g_].reshape(H, NQ, NP)[:, :, :e - s]
        pred[s:e] = pt.transpose(2, 1, 0)
    return pred, s_l
